# revision 1
# baseline (speedup 1.0000x reference)
"""Trainium2 Bass kernel for nn_DilatedGCN (gnn_message_passing).

Math (derived from the reference):
  feats F = X @ W_mlp + b_mlp                  [N, B, T, D]
  scores = concat([F[src], F[dst]]) @ W_attn + b_attn
  Per-destination-segment softmax over the DEG=8 incoming edges.
  KEY: within a segment, the dst-side term (F[dst] @ W_attn[D:] + b_attn) is
  constant, so it cancels in the softmax; and max-subtraction is unnecessary
  in f32 (scores ~ N(0, 0.7)).  Hence with
     S  = F @ W_attn[:D]       (per node)
     ES = exp(S)               (per node)
     G  = ES * F               (per node)
  each dilation graph k reduces to two gather+segment-sum ops:
     gcn_k[n] = (sum_j G[src_j]) / (sum_j ES[src_j])
  out = leaky_relu(sum_k w_k * gcn_k, 0.01) + X

Distribution: data-parallel over the 48 (b, t) pairs -> 6 per core, 8 cores.
Per core: compute bf16 tables H[n] = [ES(6*64) | G(6*64)] (1536 B rows) in
DRAM, dma_gather 16000 rows per graph in 1024-edge chunks (one chunk = 128
output nodes), segment-sum groups of 8 edges on the TensorEngine with shifted
block-diagonal one-patterns (dst ids are e//8 by construction), divide +
accumulate on Vector engine, residual via CCE-accumulate DMA.
"""

import numpy as np

B, N, T, C, D, K, DEG = 4, 2000, 12, 64, 64, 3, 8
E = N * DEG
NCORES = 8
BT = B * T              # 48
SPC = BT // NCORES      # 6 (b,t) slots per core
M = SPC * D             # 384 channels per node per core
R = N * SPC             # 12000 rows for the MLP matmuls
RCHUNK = 120            # rows per MLP matmul (multiple of SPC)
NMM = R // RCHUNK       # 100 matmul chunks
NPB = 20                # nodes per matmul chunk (RCHUNK / SPC)
ECHUNK = 1024           # edges per gather chunk (= 128 nodes)
NSLOT = (N + 127) // 128            # 16 chunks/slots per graph (last = 80)
IDXW = E // 16                      # idx columns per graph (1000)

_CACHE = {}


def _build_program(kstage=None, ksub=None, rep_main=1, rep_prol=1, tdt="bf16"):
    import os
    import concourse.bacc as bacc
    import concourse.bass as bass
    import concourse.mybir as mybir
    from concourse.tile import TileContext
    from contextlib import ExitStack

    if kstage is None:
        kstage = os.environ.get("KSTAGE", "3")
    if ksub is None:
        ksub = os.environ.get("KSUB", "gmel")

    dt = mybir.dt
    AF = mybir.ActivationFunctionType
    ALU = mybir.AluOpType
    TDT = dt.float8e4 if tdt == "fp8" else dt.bfloat16

    nc = bacc.Bacc("TRN2")

    xT1 = nc.dram_tensor("xT1", [NMM, C + 1, RCHUNK], dt.bfloat16,
                         kind="ExternalInput")
    x_rows = nc.dram_tensor("x_rows", [N, M], dt.float32, kind="ExternalInput")
    wmlpb = nc.dram_tensor("wmlpb", [C + 1, D], dt.bfloat16, kind="ExternalInput")
    wmlpbT = nc.dram_tensor("wmlpbT", [D, C + 1], dt.bfloat16, kind="ExternalInput")
    wa = nc.dram_tensor("wa", [D, D], dt.bfloat16, kind="ExternalInput")
    # 8 shifted block-diagonal segment-sum patterns: P[p, s*128 + 16s + p//8] = 1
    pshift = nc.dram_tensor("pshift", [128, 8 * 128], dt.bfloat16,
                            kind="ExternalInput")
    wkcol = nc.dram_tensor("wkcol", [128, K], dt.float32, kind="ExternalInput")
    # per (k, chunk): ECHUNK indices, 16-partition-wrapped and replicated x8
    idx = nc.dram_tensor("idx", [128, K * IDXW], dt.int16, kind="ExternalInput")
    out_rows = nc.dram_tensor("out_rows", [N, M], dt.float32, kind="ExternalOutput")
    # node tables: row n = [ES (384 ch) | G (384 ch)] bf16
    H = nc.dram_tensor("H", [N, 2 * M], TDT, kind="Internal")

    with TileContext(nc) as tc, ExitStack() as ctx:
        from concourse.library_config import mlp
        nc.gpsimd.load_library(mlp)
        const = ctx.enter_context(tc.tile_pool(name="const", bufs=1))
        ps_bf = const.tile([128, 8 * 128], dt.bfloat16)
        nc.sync.dma_start(ps_bf[:], pshift[:])
        ps_sb = ps_bf
        if TDT != dt.bfloat16:
            ps_sb = const.tile([128, 8 * 128], TDT)
            nc.vector.tensor_copy(ps_sb[:], ps_bf[:])
        wk_sb = const.tile([128, K], dt.float32)
        nc.sync.dma_start(wk_sb[:], wkcol[:])
        idx_sb = const.tile([128, K * IDXW], dt.int16)
        nc.sync.dma_start(idx_sb[:], idx[:])
        acc = const.tile([128, NSLOT * M], dt.float32)

        # ---------------- prologue: build node tables H ----------------
        pp = ctx.enter_context(tc.tile_pool(name="prol", bufs=1))
        lhsp = ctx.enter_context(tc.tile_pool(name="lhs", bufs=3))
        pps = ctx.enter_context(tc.tile_pool(name="prolps", bufs=2, space="PSUM"))

        wmlpb_sb = pp.tile([C + 1, D], dt.bfloat16)
        nc.sync.dma_start(wmlpb_sb[:], wmlpb[:])
        wmlpbT_sb = pp.tile([D, C + 1], dt.bfloat16)
        nc.sync.dma_start(wmlpbT_sb[:], wmlpbT[:])
        wa_sb = pp.tile([D, D], dt.bfloat16)
        nc.sync.dma_start(wa_sb[:], wa[:])

        # W2cat = [[W_mlp; b_mlp] | [W_mlp; b_mlp] @ Wa]  (bf16, [65, 128])
        w2ps = pps.tile([C + 1, D], dt.float32, tag="pps")
        nc.tensor.matmul(out=w2ps[:], lhsT=wmlpbT_sb[:], rhs=wa_sb[:],
                         start=True, stop=True)
        w2cat = pp.tile([C + 1, 2 * D], dt.bfloat16)
        nc.vector.tensor_copy(w2cat[:, :D], wmlpb_sb[:])
        nc.vector.tensor_copy(w2cat[:, D:], w2ps[:])

        es_bf = pp.tile([RCHUNK, NMM * D], dt.bfloat16)
        f_bf = pp.tile([RCHUNK, NMM * D], dt.bfloat16)
        for cc in [cc for _ in range(rep_prol) for cc in range(NMM // 4)]:
            fsps = pps.tile([RCHUNK, 512], dt.float32, tag="pps")
            for q in range(4):
                ch = 4 * cc + q
                lhs = lhsp.tile([C + 1, RCHUNK], dt.bfloat16, tag="lhs")
                nc.sync.dma_start(lhs[:], xT1[ch])
                nc.tensor.matmul(
                    out=fsps[:, 128 * q:128 * (q + 1)],
                    lhsT=lhs[:], rhs=w2cat[:], start=True, stop=True)
            v = fsps[:].rearrange("p (q two d) -> p q two d", q=4, two=2)
            nc.scalar.activation(
                es_bf[:, 256 * cc:256 * (cc + 1)].rearrange(
                    "p (q d) -> p q d", q=4),
                v[:, :, 1, :], AF.Exp)
            nc.vector.tensor_copy(
                f_bf[:, 256 * cc:256 * (cc + 1)].rearrange(
                    "p (q d) -> p q d", q=4),
                v[:, :, 0, :])

        g_bf = pp.tile([RCHUNK, NMM * D], TDT)
        nc.vector.tensor_tensor(g_bf[:], f_bf[:], es_bf[:], op=ALU.mult)
        es_t = es_bf
        if TDT != dt.bfloat16:
            es_t = pp.tile([RCHUNK, NMM * D], TDT)
            nc.vector.tensor_copy(es_t[:], es_bf[:])

        # write tables to DRAM:  row R = 120*chunk + p = n*6 + s
        #  -> n = 20*chunk + p//6, s = p%6; table col = 64*chunk + d
        # one DMA per a (partition group of 6): dims (s, c, d) on both sides
        for ti, (tsb, lo) in enumerate(((es_t, 0), (g_bf, M))):
            hv = H[:, lo:lo + M].rearrange("(c a) (s d) -> a s c d",
                                           a=NPB, d=D)
            for a in range(NPB):
                tv = tsb[6 * a:6 * (a + 1), :].rearrange(
                    "s (c d) -> s c d", d=D)
                eng = nc.sync if (a + ti) % 2 == 0 else nc.scalar
                eng.dma_start(hv[a], tv)

        # ---------------- main: gather + segment-sum per graph ----------------
        import os as _os
        _gb = int(_os.environ.get("KGB", "6"))
        gp = ctx.enter_context(tc.tile_pool(name="gath", bufs=_gb))
        rps = ctx.enter_context(tc.tile_pool(name="redps", bufs=6, space="PSUM"))
        ep = ctx.enter_context(tc.tile_pool(name="epi", bufs=3))

        KSTAGE, KSUB = kstage, ksub
        dummy_gth = None
        if "g" not in KSUB:
            dummy_gth = const.tile([128, 8, 2 * M], TDT)
            nc.gpsimd.memset(dummy_gth[:], 1.0)
        for k in [k for _ in range(rep_main)
                  for k in range(K if KSTAGE >= "1" else 0)]:
            for slot in range(NSLOT):
                nib = 8 if slot < NSLOT - 1 else (N - 128 * (NSLOT - 1) + 15) // 16
                ne = 128 * nib  # edges this chunk
                if "g" in KSUB:
                    gth = gp.tile([128, 8, 2 * M], TDT, tag="gth")
                    o = k * IDXW + slot * (ECHUNK // 16)
                    nc.gpsimd.dma_gather(
                        gth[:, :nib, :], H[:], idx_sb[:, o:o + ne // 16],
                        ne, ne, 2 * M, single_packet=False)
                else:
                    gth = dummy_gth
                if "m" not in KSUB:
                    continue
                denp = rps.tile([128, M], dt.float32, tag="red")
                nump = rps.tile([128, M], dt.float32, tag="red")
                for s in range(nib):
                    nc.tensor.matmul(out=denp[:],
                                     lhsT=ps_sb[:, 128 * s:128 * (s + 1)],
                                     rhs=gth[:, s, :M],
                                     start=(s == 0), stop=(s == nib - 1))
                for s in range(nib):
                    nc.tensor.matmul(out=nump[:],
                                     lhsT=ps_sb[:, 128 * s:128 * (s + 1)],
                                     rhs=gth[:, s, M:],
                                     start=(s == 0), stop=(s == nib - 1))
                if "e" not in KSUB:
                    continue
                pv = 16 * nib
                rt = ep.tile([128, M], dt.float32, tag="rt")
                nc.vector.reciprocal_approx_fast(out=rt[:pv, :],
                                                 in_=denp[:pv, :])
                accv = acc[:pv, M * slot:M * (slot + 1)]
                if k == 0:
                    nc.vector.scalar_tensor_tensor(
                        accv, rt[:pv, :], wk_sb[:pv, k:k + 1], nump[:pv, :],
                        op0=ALU.mult, op1=ALU.mult)
                else:
                    tmp = ep.tile([128, M], dt.float32, tag="tmp")
                    nc.vector.scalar_tensor_tensor(
                        tmp[:pv, :], rt[:pv, :], wk_sb[:pv, k:k + 1],
                        nump[:pv, :], op0=ALU.mult, op1=ALU.mult)
                    nc.vector.tensor_tensor(accv, accv, tmp[:pv, :],
                                            op=ALU.add)

        # ---------------- final: leaky relu + store, residual via DMA ---------
        for slot in range(NSLOT if KSTAGE >= "2" else 0):
            pv = 128 if slot < NSLOT - 1 else N - 128 * (NSLOT - 1)
            ot = ep.tile([128, M], dt.float32, tag="out")
            accv = acc[:pv, M * slot:M * (slot + 1)]
            # leaky_relu(x, .01) = .01*x + .99*relu(x) = .01*x + relu(.99*x)
            rl = ep.tile([128, M], dt.float32, tag="rl")
            nc.scalar.activation(rl[:pv, :], accv, AF.Relu, scale=0.99)
            nc.vector.scalar_tensor_tensor(ot[:pv, :], accv, 0.01, rl[:pv, :],
                                           op0=ALU.mult, op1=ALU.add)
            base = 128 * slot
            if KSTAGE >= "3":
                # residual: ot += x_rows rows (CCE accumulate into SBUF)
                nc.gpsimd.dma_start(ot[:pv, :], x_rows[base:base + pv, :],
                                    accum_op=ALU.add)
            nc.sync.dma_start(out_rows[base:base + pv, :], ot[:pv, :])

    nc.compile()
    return nc


def _get_program(kstage=None, ksub=None, rep_main=1, rep_prol=1, tdt=None):
    import os
    if tdt is None:
        tdt = os.environ.get("KTDT", "bf16")
    key = ("nc", kstage, ksub, rep_main, rep_prol, tdt)
    if key not in _CACHE:
        _CACHE[key] = _build_program(kstage, ksub, rep_main, rep_prol, tdt)
    return _CACHE[key]


def _prep_inputs(input_feature, W_mlp, b_mlp, W_attn, b_attn, weight, edges):
    import ml_dtypes
    bf16 = ml_dtypes.bfloat16

    X = np.asarray(input_feature, dtype=np.float32)
    src = np.asarray(edges)[:, 0, :].astype(np.int64)
    dst = np.asarray(edges)[:, 1, :]
    exp_dst = np.broadcast_to(np.repeat(np.arange(N, dtype=dst.dtype), DEG), (K, E))
    assert np.array_equal(dst, exp_dst), "kernel assumes dst = repeat(arange(N), 8)"
    assert src.min() >= 0 and src.max() < N

    A = np.concatenate([np.asarray(W_mlp, np.float32),
                        np.asarray(b_mlp, np.float32)[None, :]], axis=0)  # [65, 64]
    Wa = np.asarray(W_attn, np.float32)[:D, :]                             # [64, 64]
    wmlpb_h = np.ascontiguousarray(A.astype(bf16))
    wmlpbT_h = np.ascontiguousarray(A.T.astype(bf16))
    wa_h = np.ascontiguousarray(Wa.astype(bf16))

    # pshift[p, 128*s + q] = 1 iff q == 16*s + p//8
    ps = np.zeros((128, 8, 128), np.float32)
    p_ar = np.arange(128)
    for s in range(8):
        ps[p_ar, s, 16 * s + p_ar // 8] = 1.0
    pshift_h = np.ascontiguousarray(ps.reshape(128, 1024).astype(bf16))
    wk = np.asarray(weight, np.float32).reshape(K)
    wkcol_h = np.ascontiguousarray(
        np.broadcast_to(wk[None, :], (128, K)).astype(np.float32))

    # per graph: wrap each 16-index group [g, 16] -> [16, g], replicate x8
    blocks = []
    for k in range(K):
        blocks.append(np.tile(src[k].reshape(IDXW, 16).T, (8, 1)))
    idx_h = np.ascontiguousarray(np.concatenate(blocks, axis=1).astype(np.int16))

    # per-core slices: slot = b*T + t; core c owns slots [6c, 6c+6)
    Xn = np.transpose(X, (1, 0, 2, 3)).reshape(N, BT, C)
    in_maps = []
    for c in range(NCORES):
        Xloc = Xn[:, SPC * c:SPC * (c + 1), :]                   # [N, 6, C]
        x_rows_h = np.ascontiguousarray(Xloc.reshape(N, M))
        Xr = Xloc.reshape(R, C)
        # [NMM, C+1, RCHUNK]: chunk-major transposed rows + ones row
        xT1_h = np.empty((NMM, C + 1, RCHUNK), dtype=bf16)
        xT1_h[:, :C, :] = Xr.reshape(NMM, RCHUNK, C).transpose(0, 2, 1).astype(bf16)
        xT1_h[:, C, :] = np.asarray(1.0, dtype=bf16)
        in_maps.append({
            "xT1": np.ascontiguousarray(xT1_h),
            "x_rows": x_rows_h,
            "wmlpb": wmlpb_h,
            "wmlpbT": wmlpbT_h,
            "wa": wa_h,
            "pshift": pshift_h,
            "wkcol": wkcol_h,
            "idx": idx_h,
        })
    return in_maps


def _assemble_output(results):
    out_all = np.empty((N, BT, C), dtype=np.float32)
    for c in range(NCORES):
        out_all[:, SPC * c:SPC * (c + 1), :] = \
            results[c]["out_rows"].reshape(N, SPC, C)
    return np.ascontiguousarray(
        out_all.reshape(N, B, T, C).transpose(1, 0, 2, 3))


def kernel(input_feature, W_mlp, b_mlp, W_attn, b_attn, weight, edges,
           _trace=False, **trace_kwargs):
    from concourse.bass_utils import run_bass_kernel_spmd

    in_maps = _prep_inputs(input_feature, W_mlp, b_mlp, W_attn, b_attn,
                           weight, edges)
    nc = _get_program()
    res = run_bass_kernel_spmd(nc, in_maps, list(range(NCORES)),
                               trace=_trace, **trace_kwargs)
    out = _assemble_output(res.results)
    if _trace:
        return out, res
    return out

